# revision 62
# baseline (speedup 1.0000x reference)
"""GQA kernel v6 for TRN2, 8 NeuronCores.

Sharding: DP2 (batch) x TP4 (head groups). Core r handles batch b=r//4,
quad q=r%4 -> global Q heads 8q..8q+7 (KV groups 2q, 2q+1).

v6 vs v4 (hw-ablation driven; ~628us -> ~519us on the same meter):
  - x^T resident in SBUF: 16 [128,2048] tiles loaded once per rep
    (v4 re-loaded x per chunk: 64 descriptors, 2x traffic). HW ablation:
    P1 projections 179us -> 120us.
  - Weights via big staged DMAs (host pre-packs tiles contiguously,
    KV tiles first) + on-chip DVE scatter to offset-0 stationary tiles.
    Per-tile weight DMAs measured +52us worse on this hw (HWDGE
    descriptor cost is real); column-sliced stationaries are 10-20x
    worse still (v4 note).
  - P1: one K/V sweep over k (8 PSUM banks = K,V x 4 chunks), V'
    transposes via XBAR dma_start_transpose on the ACT hwdge queue
    (PE/PSUM-free, and it keeps the SP queue clear for x loads), then
    2 Q chunk-pair passes reusing the same 8 banks (no pool churn).
  - P4(c) matmuls drip 2-per-k-iter into att(c+2) through a dedicated
    PSUM bank, each with an ordering-only dep on its k-iter's first
    scores matmul so the tile scheduler cannot hoist gather-gated work
    ahead of ready attention matmuls on the in-order PE queue (v4
    stalled ~50us/chunk this way in the timeline sim). ci input is a
    single 1-descriptor DMA on the gpsimd queue (cannot block the SP
    store stream).
  - One reciprocal op over all 128 partitions (DVE charges free-dim
    only); yt stores 1 descriptor per chunk.

PSUM in att: s0/s1 f32 [128,1024] (4) + poA/poB (2) + sm (1) + py (1).
Measured (R=25 meter): P1 120us, +scores/PV 156us (ACT-bound chunks),
+exp/esum 99us, +softmax tails 46us, +P4 pipeline 99us, collectives
~hidden (noag saves only 18us).
"""

import numpy as np

import concourse.bacc as bacc
import concourse.bass as bass
import concourse.mybir as mybir
import concourse.tile as tile
from concourse.bass_utils import run_bass_kernel_spmd
from concourse.masks import make_identity
from concourse.tile_rust import add_dep_helper

D = 2048          # d_model
H = 32            # query heads
G = 8             # kv groups
DK = 64           # head dim
B = 2
S = 2048          # tokens per batch
TOK = S
NCORES = 8
NQ = 4            # quads (TP degree)
LH = 8            # local query heads per core
QDIM = LH * DK    # 512 local q dims
KVDIM = 2 * DK    # 128 local kv dims (2 groups)
NKT = D // 128    # 16 contraction tiles over d_model
NTT = TOK // 128  # 16 token tiles of 128
NC512 = TOK // 512  # 4 token chunks of 512

F32 = mybir.dt.float32
F16 = mybir.dt.float16
EXP = mybir.ActivationFunctionType.Exp


def _build_nc(repeat: int = 1, stages: str = "full",
              p4_depth: int = 2, xres: bool = True,
              wsmall: bool = False) -> bass.Bass:
    """stages: 'p1' (projections only), 'nop4' (no out-proj/ci/yt),
    'noag' (no collective, P4 reads stale ct_all), 'full'.
    p4_depth: 1 = drip P4(c) into att(c+1), 2 = into att(c+2).
    xres: keep x^T resident in SBUF (loaded once, 8MB).
    wsmall: per-tile weight DMAs instead of big staging DMA+scatter."""
    nc = bacc.Bacc("TRN2", num_devices=NCORES)

    xt_d = nc.dram_tensor("xt", [D, TOK], F16, kind="ExternalInput")
    # host packs wqkv tiles (k,m) contiguously: [128, (6k+m)*128 + c]
    wqkv_d = nc.dram_tensor("wqkv", [128, NKT * 6 * 128], F16,
                            kind="ExternalInput")
    bqkv_d = nc.dram_tensor("bqkv", [6 * 128], F32, kind="ExternalInput")
    # host packs wo tiles (k,m): [128, (4k+m)*128 + c]
    wo_d = nc.dram_tensor("wo", [128, NKT * 4 * 128], F16,
                          kind="ExternalInput")
    bo_d = nc.dram_tensor("bo", [QDIM], F32, kind="ExternalInput")
    # yt layout: [128, c*2048 + m*512 + q] (host unpacks)
    yt_d = nc.dram_tensor("yt", [128, NC512 * 4 * 512], F32,
                          kind="ExternalOutput")

    ct_src = [nc.dram_tensor(f"ct_src{c}", [QDIM, 512], F16)
              for c in range(NC512)]
    ct_all = [nc.dram_tensor(f"ct_all{c}", [D, 512], F16)
              for c in range(NC512)]
    replica_groups = [[0, 1, 2, 3], [4, 5, 6, 7]]

    from contextlib import ExitStack
    with tile.TileContext(nc) as tc:
        with ExitStack() as _stk:
            persist = _stk.enter_context(tc.tile_pool(name="persist", bufs=1))
            xsp = _stk.enter_context(tc.tile_pool(name="xs", bufs=3))
            wstp = _stk.enter_context(tc.tile_pool(name="wst", bufs=1))
            wpool = _stk.enter_context(tc.tile_pool(name="wq", bufs=1))
            wop = _stk.enter_context(tc.tile_pool(name="wo", bufs=1))
            qtp = _stk.enter_context(tc.tile_pool(name="qt", bufs=1))
            ktvp = _stk.enter_context(tc.tile_pool(name="ktv", bufs=1))
            vtpool = _stk.enter_context(tc.tile_pool(name="vt", bufs=1))
            stp = _stk.enter_context(tc.tile_pool(name="st", bufs=4))
            esump = _stk.enter_context(tc.tile_pool(name="esum", bufs=2))
            nrmp = _stk.enter_context(tc.tile_pool(name="nrm", bufs=2))
            ctxp = _stk.enter_context(tc.tile_pool(name="ctx", bufs=2))
            cinp = _stk.enter_context(
                tc.tile_pool(name="cin", bufs=1 if xres else 2))
            youtp = _stk.enter_context(
                tc.tile_pool(name="yout", bufs=1 if xres else 2))
            if xres:
                xrp = _stk.enter_context(tc.tile_pool(name="xr", bufs=1))

            # ---- persistent SBUF ----
            qt = [qtp.tile([128, TOK], F16, tag=f"qt{t}", name=f"qt{t}")
                  for t in range(4)]
            ktk = [ktvp.tile([128, 128], F16, tag=f"kt{k}", name=f"kt{k}")
                   for k in range(NTT)]
            vpk = [[ktvp.tile([128, DK], F16, tag=f"vp{j}_{k}",
                              name=f"vp{j}_{k}") for k in range(NTT)]
                   for j in range(2)]
            ones_col = ktvp.tile([128, 1], F16, tag="onec", name="onec")
            bias6 = persist.tile([128, 6], F32)
            bo_t = persist.tile([128, 4], F32)
            ones16 = persist.tile([128, 64], F16)
            ident = persist.tile([128, 64], F16)

            nc.vector.memset(ones16[:], 1.0)
            nc.vector.memset(ones_col[:], 1.0)
            make_identity(nc, ident[0:64, :])
            make_identity(nc, ident[64:128, :])
            for m in range(6):
                nc.sync.dma_start(bias6[:, m : m + 1],
                                  bqkv_d[bass.ts(m, 128)].unsqueeze(1))
            for m in range(4):
                nc.sync.dma_start(bo_t[:, m : m + 1],
                                  bo_d[bass.ts(m, 128)].unsqueeze(1))

            for _rep in range(repeat):
                # ---------------- P1: projections ----------------
                # host packs KV tiles first: col tile (2k + m-4) for m in
                # (4,5), then Q tiles at 32*128 + (4k+m)*128
                # staged in rounds of 32 tiles (8KB staging) so the
                # staging buffer stays small with x resident
                w_tiles = [[None] * 6 for _ in range(NKT)]
                kvq_order = ([(k, m) for k in range(NKT)
                              for m in (4, 5)]
                             + [(k, m) for k in range(NKT)
                                for m in range(4)])

                def stage_scatter(dram, order, dest, pool, pfx, base=0):
                    for lo in range(0, len(order), 32):
                        n = min(32, len(order) - lo)
                        wstg = wstp.tile([128, 32 * 128], F16,
                                         tag="wstg", name="wstg")
                        nc.sync.dma_start(
                            wstg[:, 0 : n * 128],
                            dram[:, (base + lo) * 128 :
                                 (base + lo + n) * 128])
                        for j in range(n):
                            k, m = order[lo + j]
                            wt = pool.tile([128, 128], F16,
                                           tag=f"{pfx}{k}_{m}",
                                           name=f"{pfx}{k}_{m}")
                            nc.vector.tensor_copy(
                                wt[:], wstg[:, bass.ts(j, 128)])
                            dest[k][m] = wt

                # KV weight tiles first, then all x loads, THEN the Q
                # weight staging: keeps the 2MB Q staging DMA off the SP
                # queue's critical path into the K/V sweep
                stage_scatter(wqkv_d, kvq_order[:32], w_tiles, wpool, "w")
                xr = [None] * NKT
                if xres:
                    for k in range(NKT):
                        xr[k] = xrp.tile([128, TOK], F16, tag=f"xr{k}",
                                         name=f"xr{k}")
                        nc.sync.dma_start(xr[k][:],
                                          xt_d[bass.ts(k, 128), :])
                stage_scatter(wqkv_d, kvq_order[32:], w_tiles, wpool,
                              "w", base=32)

                vt_sb = vtpool.tile([128, TOK], F16, tag="vt", name="vt")

                # shared 8-bank pool: KV sweep, then Q passes reuse tags
                with tc.tile_pool(name="pp", bufs=1, space="PSUM") as pp:
                    def pbank(i):
                        return pp.tile([128, 512], F32, tag=f"p{i}",
                                       name=f"p{i}")
                    psk = [pbank(c) for c in range(NC512)]
                    psv = [pbank(4 + c) for c in range(NC512)]
                    for k in range(NKT):
                        if xres:
                            xk = xr[k]
                        else:
                            xk = xsp.tile([128, TOK], F16, tag="xkv",
                                          name="xkv")
                            nc.sync.dma_start(xk[:],
                                              xt_d[bass.ts(k, 128), :])
                        for c in range(NC512):
                            nc.tensor.matmul(
                                psk[c][:], w_tiles[k][4][:],
                                xk[:, bass.ts(c, 512)],
                                start=(k == 0), stop=(k == NKT - 1))
                            nc.tensor.matmul(
                                psv[c][:], w_tiles[k][5][:],
                                xk[:, bass.ts(c, 512)],
                                start=(k == 0), stop=(k == NKT - 1))
                    for c in range(NC512):
                        for kk in range(4):
                            nc.vector.tensor_scalar_add(
                                ktk[4 * c + kk][:],
                                psk[c][:, bass.ts(kk, 128)], bias6[:, 4:5])
                        nc.vector.tensor_scalar_add(
                            vt_sb[:, bass.ts(c, 512)], psv[c][:],
                            bias6[:, 5:6])

                    # V' via XBAR dma transpose (no PSUM/PE involved);
                    # on the ACT hwdge queue so the SP queue stays free
                    # for the Q-pass x loads
                    for j in range(2):
                        for tt in range(NTT):
                            nc.scalar.dma_start_transpose(
                                vpk[j][tt][:],
                                vt_sb[bass.ts(j, 64), bass.ts(tt, 128)])

                    # Q: 2 chunk-pair passes reusing the same 8 banks
                    for cp in range(2):
                        psq = [pbank(2 * m + cc)
                               for m in range(4) for cc in range(2)]
                        for k in range(NKT):
                            if xres:
                                def xsl(cc, k=k, cp=cp):
                                    lo = 1024 * cp + 512 * cc
                                    return xr[k][:, lo : lo + 512]
                            else:
                                xqt = xsp.tile([128, 1024], F16,
                                               tag="xq", name="xq")
                                nc.sync.dma_start(
                                    xqt[:],
                                    xt_d[bass.ts(k, 128),
                                         bass.ts(cp, 1024)])
                                def xsl(cc, xqt=xqt):
                                    return xqt[:, bass.ts(cc, 512)]
                            for m in range(4):
                                for cc in range(2):
                                    nc.tensor.matmul(
                                        psq[2 * m + cc][:],
                                        w_tiles[k][m][:],
                                        xsl(cc),
                                        start=(k == 0),
                                        stop=(k == NKT - 1))
                        for m in range(4):
                            for cc in range(2):
                                nc.vector.tensor_scalar_add(
                                    qt[m][:, bass.ts(2 * cp + cc, 512)],
                                    psq[2 * m + cc][:], bias6[:, m : m + 1])

                if stages == "p1":
                    continue

                wo_tiles = [[None] * 4 for _ in range(NKT)]
                if stages != "nop4":
                    # wo staged+scattered; first needed at P4(0)
                    wo_order = [(k, m) for k in range(NKT)
                                for m in range(4)]
                    stage_scatter(wo_d, wo_order, wo_tiles, wop, "wo")

                # ---------------- P2 + P4, chunk-pipelined ----------------
                with ExitStack() as _s2:
                    psc = _s2.enter_context(
                        tc.tile_pool(name="psc", bufs=1, space="PSUM"))
                    pov = _s2.enter_context(
                        tc.tile_pool(name="pov", bufs=1, space="PSUM"))
                    psm = _s2.enter_context(
                        tc.tile_pool(name="psm", bufs=1, space="PSUM"))
                    pyp = _s2.enter_context(
                        tc.tile_pool(name="py", bufs=1, space="PSUM"))

                    def p4_work(c4):
                        """ci DMA now (gpsimd queue), return 64 matmul
                        closures to drip into att k-iters."""
                        ci = cinp.tile([128, NKT * 512], F16, tag="ci",
                                       name="ci")
                        yo = youtp.tile([128, 2048], F32, tag="yo",
                                        name="yo")
                        nc.gpsimd.dma_start(
                            ci[:].rearrange("p (k q) -> p k q", k=NKT),
                            ct_all[c4][:, :].rearrange(
                                "(k p) q -> p k q", p=128))
                        psy = [None]
                        def mk_mm(m, k):
                            def f(anchor=None):
                                if k == 0:
                                    psy[0] = pyp.tile([128, 512], F32,
                                                      tag="py", name="py")
                                mm = nc.tensor.matmul(
                                    psy[0][:], wo_tiles[k][m][:],
                                    ci[:, bass.ts(k, 512)],
                                    start=(k == 0), stop=(k == NKT - 1))
                                if anchor is not None:
                                    # ordering-only edge: keep gather-gated
                                    # P4 work from being hoisted ahead of
                                    # ready attention matmuls on the
                                    # in-order PE queue
                                    add_dep_helper(
                                        mm.ins, anchor.ins, sync=False,
                                        reason="p4 drips after att")
                                if k == NKT - 1:
                                    nc.vector.tensor_scalar_add(
                                        yo[:, bass.ts(m, 512)], psy[0][:],
                                        bo_t[:, m : m + 1])
                                    if m == 3:
                                        nc.sync.dma_start(
                                            yt_d[:, bass.ts(c4, 2048)],
                                            yo[:])
                            return f
                        return [mk_mm(m, k) for m in range(4)
                                for k in range(NKT)]

                    # depth-2 pipeline: P4(c) drips 2-per-iter into
                    # att(c+2)
                    inflight_p4 = []
                    fresh_p4 = []

                    fixed_est = None
                    if stages == "attpe":
                        fixed_est = [stp.tile([128, 1024], F16,
                                              tag=f"fe{i}", name=f"fe{i}")
                                     for i in range(2)]
                        for fe in fixed_est:
                            nc.vector.memset(fe[:], 0.25)

                    attmin = stages in ("attpe", "attnosm")
                    lastc = NC512 - 1
                    for nc5 in range(NC512):
                        for half in range(2):
                            # last chunk: drain P4(lastc-2) in half0 at
                            # 4/iter, then pull P4(lastc-1) forward into
                            # half1 (its gather finished early in this
                            # chunk) so the tail holds only P4(lastc)
                            if nc5 == lastc and half == 1 and fresh_p4:
                                inflight_p4.extend(fresh_p4)
                                fresh_p4 = []
                            poA = pov.tile([128, 512], F32, tag="poA",
                                           name="poA")
                            poB = pov.tile([128, 512], F32, tag="poB",
                                           name="poB")
                            esm = (None if attmin else
                                   esump.tile([128, 2048], F16, tag="es",
                                              name="es"))
                            prev_pv = None
                            for k in range(NTT):
                                # scores: slot i tile, row-pack j -> +512j
                                est = []
                                anchor = None
                                for i in range(2):
                                    t = 2 * half + i
                                    s2 = psc.tile([128, 1024], F32,
                                                  tag=f"s{i}", name=f"s{i}")
                                    mm0 = nc.tensor.matmul(
                                        s2[:, 0:512],
                                        ktk[k][0:64, :],
                                        qt[t][0:64, bass.ts(nc5, 512)],
                                        start=True, stop=True,
                                        tile_position=(0, 0))
                                    if anchor is None:
                                        anchor = mm0
                                    nc.tensor.matmul(
                                        s2[:, 512:1024],
                                        ktk[k][64:128, :],
                                        qt[t][64:128, bass.ts(nc5, 512)],
                                        start=True, stop=True,
                                        tile_position=(64, 0))
                                    if stages == "attpe":
                                        est.append(fixed_est[i])
                                        continue
                                    e2 = stp.tile([128, 1024], F16,
                                                  tag=f"e{i}", name=f"e{i}")
                                    nc.scalar.activation(e2[:], s2[:], EXP)
                                    est.append(e2)
                                    if attmin:
                                        continue
                                    if k == 0:
                                        nc.vector.tensor_copy(
                                            esm[:, bass.ts(i, 1024)], e2[:])
                                    else:
                                        nc.vector.tensor_add(
                                            esm[:, bass.ts(i, 1024)],
                                            esm[:, bass.ts(i, 1024)], e2[:])
                                # PV for previous k (software pipeline)
                                if prev_pv is not None:
                                    prev_pv()
                                def mk_pv(e0=est[0], e1=est[1], k=k):
                                    def f():
                                        vk0 = vpk[0][k][:]
                                        vk1 = vpk[1][k][:]
                                        st_ = (k == 0)
                                        sp_ = (k == NTT - 1)
                                        nc.tensor.matmul(
                                            poA[0:64, :], vk0,
                                            e0[:, 0:512],
                                            start=st_, stop=sp_,
                                            tile_position=(0, 0))
                                        nc.tensor.matmul(
                                            poA[64:128, :], vk1,
                                            e1[:, 512:1024],
                                            start=st_, stop=sp_,
                                            tile_position=(0, 64))
                                        nc.tensor.matmul(
                                            poB[0:64, :], vk0,
                                            e1[:, 0:512],
                                            start=st_, stop=sp_,
                                            tile_position=(0, 0))
                                        return nc.tensor.matmul(
                                            poB[64:128, :], vk1,
                                            e0[:, 512:1024],
                                            start=st_, stop=sp_,
                                            tile_position=(0, 64))
                                    return f
                                prev_pv = mk_pv()
                                # drip-feed P4 matmuls, held behind this
                                # iter's first scores matmul
                                for _ in range(4 if nc5 == lastc else 2):
                                    if inflight_p4:
                                        inflight_p4.pop(0)(anchor)
                            last_pv = prev_pv()
                            if attmin:
                                continue

                            # sums: M=1 matmuls into sm parts 0/32/64/96
                            sm = psm.tile([128, 512], F32, tag="sm",
                                          name="sm")
                            slot_ij = [(0, 0), (1, 1), (1, 0), (0, 1)]
                            for s, (i, j) in enumerate(slot_ij):
                                off = 1024 * i + 512 * j
                                nc.tensor.matmul(
                                    sm[32 * s : 32 * s + 1, :],
                                    ones_col[:],
                                    esm[:, off : off + 512],
                                    start=True, stop=True,
                                    tile_position=(0, 32 * s))
                            rcpT = nrmp.tile([128, 512], F16, tag="rcp",
                                             name="rcp")
                            # one op over all partitions costs the same as
                            # one row on DVE (free-dim cycles); only rows
                            # 0/32/64/96 are consumed downstream
                            with nc.allow_low_precision(
                                    reason="softmax denom f16"):
                                nc.vector.reciprocal(rcpT[:], sm[:])
                            cxA = nrmp.tile([128, 512], F32, tag="cxA",
                                            name="cxA")
                            cxB = nrmp.tile([128, 512], F32, tag="cxB",
                                            name="cxB")
                            nc.vector.tensor_copy(cxA[:], poA[:])
                            nc.vector.tensor_copy(cxB[:], poB[:])
                            for s, (bank, lo) in enumerate(
                                    [(poA, 0), (poA, 64), (poB, 0),
                                     (poB, 64)]):
                                nc.tensor.matmul(
                                    bank[lo : lo + 64, :],
                                    ones16[32 * s : 32 * s + 1, :],
                                    rcpT[32 * s : 32 * s + 1, :],
                                    start=True, stop=True,
                                    tile_position=(32 * s, lo))
                            for bank, cx, sl, sh in ((poA, cxA, 0, 1),
                                                     (poB, cxB, 2, 3)):
                                ct_t = ctxp.tile([128, 512], F16, tag="ct",
                                                 name="ct")
                                nc.vector.tensor_mul(ct_t[:], cx[:],
                                                     bank[:])
                                for s, lo in ((sl, 0), (sh, 64)):
                                    i, j = slot_ij[s]
                                    lhead = 2 * half + i + 4 * j
                                    nc.sync.dma_start(
                                        ct_src[nc5][bass.ts(lhead, 64), :],
                                        ct_t[lo : lo + 64, :])
                        if attmin:
                            continue
                        if stages != "noag":
                            nc.gpsimd.collective_compute(
                                "AllGather", mybir.AluOpType.bypass,
                                replica_groups=replica_groups,
                                ins=[ct_src[nc5][:]],
                                outs=[ct_all[nc5][:]])
                        if stages != "nop4":
                            while inflight_p4:
                                inflight_p4.pop(0)(last_pv)
                            inflight_p4 = fresh_p4
                            fresh_p4 = p4_work(nc5)
                    for work in (inflight_p4, fresh_p4):
                        while work:
                            work.pop(0)(last_pv)

    nc.compile()
    return nc


_NC_CACHE = {}


def _get_nc(repeat: int = 1):
    if repeat not in _NC_CACHE:
        _NC_CACHE[repeat] = _build_nc(repeat)
    return _NC_CACHE[repeat]


def _prep_core_inputs(x, Wq, bq, Wk, bk, Wv, bv, Wo, bo, core):
    b, q = divmod(core, NQ)
    xt = np.ascontiguousarray(x[b].T).astype(np.float16)  # [D, TOK]

    # local head order: pairs (t, t+4) interleaved -> [0,4,1,5,2,6,3,7]
    head_order = [0, 4, 1, 5, 2, 6, 3, 7]
    qcols = []
    for L in head_order:
        gh = 8 * q + L
        qcols.extend(range(gh * DK, (gh + 1) * DK))
    kv0 = 2 * q * DK
    kvcols = list(range(kv0, kv0 + 2 * DK))

    wqkv = np.empty((D, 6 * 128), dtype=np.float16)
    wqkv[:, :QDIM] = (Wq[:, qcols] / 8.0).astype(np.float16)
    wqkv[:, QDIM : QDIM + KVDIM] = Wk[:, kvcols].astype(np.float16)
    wqkv[:, QDIM + KVDIM :] = Wv[:, kvcols].astype(np.float16)
    # pack KV tiles first (col tile 2k+m-4), then Q tiles (32 + 4k+m)
    wt4 = wqkv.reshape(NKT, 128, 6, 128)           # [k, p, m, c]
    kv_part = wt4[:, :, 4:6, :].transpose(1, 0, 2, 3).reshape(128, -1)
    q_part = wt4[:, :, 0:4, :].transpose(1, 0, 2, 3).reshape(128, -1)
    wqkv_p = np.ascontiguousarray(
        np.concatenate([kv_part, q_part], axis=1))
    bqkv = np.concatenate([bq[qcols] / 8.0, bk[kvcols], bv[kvcols]]
                          ).astype(np.float32)

    out_lo = 512 * q
    wo = np.ascontiguousarray(Wo[:, out_lo : out_lo + QDIM]).astype(
        np.float16)
    wo_p = np.ascontiguousarray(
        wo.reshape(NKT, 128, 4, 128).transpose(1, 0, 2, 3)
        .reshape(128, NKT * 4 * 128))
    bo_s = np.ascontiguousarray(bo[out_lo : out_lo + QDIM]).astype(
        np.float32)

    return {"xt": xt, "wqkv": wqkv_p, "bqkv": bqkv, "wo": wo_p, "bo": bo_s}


def kernel(x, Wq, bq, Wk, bk, Wv, bv, Wo, bo, _trace=False):
    args = [np.asarray(a, dtype=np.float32)
            for a in (x, Wq, bq, Wk, bk, Wv, bv, Wo, bo)]
    nc = _get_nc()
    in_maps = [_prep_core_inputs(*args, core) for core in range(NCORES)]
    res = run_bass_kernel_spmd(nc, in_maps, core_ids=list(range(NCORES)),
                               trace=_trace)

    y = np.empty((B, S, D), dtype=np.float32)
    for core in range(NCORES):
        b, q = divmod(core, NQ)
        # yt layout [128, c*2048 + m*512 + tq]
        yt = res.results[core]["yt"].reshape(128, NC512, 4, 512)
        blk = yt.transpose(1, 3, 2, 0).reshape(S, QDIM)
        y[b, :, 512 * q : 512 * (q + 1)] = blk
    if _trace:
        return y, res
    return y
